# revision 28
# baseline (speedup 1.0000x reference)
"""Bass/Trainium2 kernel for BiasedMultiheadAttention (v3).

Problem shapes (hardcoded): B=2, L=2048, D=1024, H=16, d=64.
Sharding: 8 cores = 2 batches x 4 head-groups (4 heads per core).
Each core computes its heads' attention and a partial out-projection;
host sums the 4 partials per batch (fp32) and adds b_out.

v3 structure (per core, heads h0..h0+3, head-pairs hp in {0,1}):
  bias     : host precomputes exp(attn_bias) * (1 - pad) in bf16 S^T tiles;
             exp(qk + bias) = exp(qk) * exp(bias) -> no PSUM bias preload,
             no pad bias on the ACT exp
  QK^T     : S^T[k,q] pairs (K=64 at rows 0-63 / 64-127) into a
             double-buffered [128,2,512] PSUM pool tile per kc (separate
             pool tiles, NOT one ring tile: Tile dep-tracking is
             tile-granular and a shared ring serializes QK against exp)
  softmax  : per-kc ACT exp (N=1024, one call per kc);
             DVE multiply prob = e_qk * e_bias
  AV + Z   : aug-v (ones column) M=65 matmuls accumulate O^T rows 0-63 and
             the denominator Z in row 64 of each head's bank
  norm     : DVE reciprocal + tensor_mul read the ot banks directly from
             PSUM; 1/Z broadcast via K=1 ones-matmuls into the shared
             pp bank; odd head's otn SBUF->SBUF DMA'd to partitions 64:127
  out-proj : K=128 (head-pair) contraction, 2 matmuls per tile, bf16 partial
  schedule : all cross-phase work (epilogue, out-proj, v in-proj, late qk
             in-proj) is emitted as closures injected one-per-kc-iteration
             into a later phase so no engine sits behind a serial prologue
"""

import numpy as np
import ml_dtypes

B, L, D, H = 2, 2048, 1024, 16
NHC = 4          # heads per core
d = 64
QB = 512         # query block (matmul moving free dim)

_BF16 = ml_dtypes.bfloat16

_cached = {}


def _build_nc(Lx=L, loop_n=1):
    import contextlib

    import concourse.bacc as bacc
    import concourse.mybir as mybir
    import concourse.tile as tile

    fp32 = mybir.dt.float32
    bf16 = mybir.dt.bfloat16
    Exp = mybir.ActivationFunctionType.Exp
    Ident = mybir.ActivationFunctionType.Identity

    nqb = Lx // QB
    nkc = Lx // 128
    nlc = Lx // 128

    nc = bacc.Bacc("TRN2", target_bir_lowering=False)

    xT_d = nc.dram_tensor("xT", [D, Lx], bf16, kind="ExternalInput")
    wqkT_d = nc.dram_tensor("wqkT", [D, 512], bf16, kind="ExternalInput")
    wvT_d = nc.dram_tensor("wvT", [D, 256], bf16, kind="ExternalInput")
    wo2_d = nc.dram_tensor("wo2", [128, 2, D], bf16, kind="ExternalInput")
    bqk_d = nc.dram_tensor("bqk", [128, 4], fp32, kind="ExternalInput")
    bvr_d = nc.dram_tensor("bvr", [128, 256], fp32, kind="ExternalInput")
    biasT_d = nc.dram_tensor("biasT", [2, Lx // 128, Lx // QB, 128, 2, QB], bf16, kind="ExternalInput")
    out_d = nc.dram_tensor("partial", [Lx, D], bf16, kind="ExternalOutput")

    with tile.TileContext(nc) as tc:
        with contextlib.ExitStack() as ctx:
            const = ctx.enter_context(tc.tile_pool(name="const", bufs=1))
            biasp = ctx.enter_context(tc.tile_pool(name="biasp", bufs=16))
            expp = ctx.enter_context(tc.tile_pool(name="expp", bufs=4))
            probsp = ctx.enter_context(tc.tile_pool(name="probsp", bufs=7))
            zrecp = ctx.enter_context(tc.tile_pool(name="zrecp", bufs=2))
            outp = ctx.enter_context(tc.tile_pool(name="outp", bufs=4))
            # 8 PSUM banks: QK double-buffer 4 + AV accumulators 2 + pp 2
            ps_sp = ctx.enter_context(tc.tile_pool(name="ps_sp", bufs=2, space="PSUM"))
            ps_ot = ctx.enter_context(tc.tile_pool(name="ps_ot", bufs=2, space="PSUM"))
            ps_pp = ctx.enter_context(tc.tile_pool(name="ps_pp", bufs=2, space="PSUM"))

            def _emit():
                # ---- persistent SBUF ----
                xT_sb = const.tile([128, 8, Lx], bf16, name="xT_sb", tag="xT_sb")
                xT_r = xT_d.rearrange("(dc p) l -> p dc l", p=128)
                for dc in range(8):
                    nc.sync.dma_start(xT_sb[:, dc, :], xT_r[:, dc, :])
                wqkT_sb = const.tile([128, 8, 512], bf16, name="wqkT_sb", tag="wqkT_sb")
                nc.sync.dma_start(
                    wqkT_sb[:], wqkT_d.rearrange("(dc p) f -> p dc f", p=128)
                )
                wvT_sb = const.tile([128, 8, 256], bf16, name="wvT_sb", tag="wvT_sb")
                nc.sync.dma_start(
                    wvT_sb[:], wvT_d.rearrange("(dc p) f -> p dc f", p=128)
                )
                wo2_sb = const.tile([128, 2, D], bf16, name="wo2_sb", tag="wo2_sb")
                nc.sync.dma_start(wo2_sb[:], wo2_d[:])
                bqk_sb = const.tile([128, 4], fp32, name="bqk_sb", tag="bqk_sb")
                nc.sync.dma_start(bqk_sb[:], bqk_d[:])
                bvr_sb = const.tile([128, 256], fp32, name="bvr_sb", tag="bvr_sb")
                nc.sync.dma_start(bvr_sb[:], bvr_d[:])

                qkT_sb = const.tile([128, 4, Lx], bf16, name="qkT_sb", tag="qkT_sb")
                v_sb = const.tile([128, nlc, 4, 65], bf16, name="v_sb", tag="v_sb")
                nc.vector.memset(v_sb[:, :, :, 64:65], 1.0)
                otn2_sb = const.tile([128, 2, Lx], bf16, name="otn2_sb", tag="otn2_sb")
                ones_f32 = const.tile([65, 64], fp32, name="ones_f32", tag="ones_f32")
                nc.vector.memset(ones_f32[:], 1.0)
                # preload the exp table set while DMAs stream in
                warm = zrecp.tile([65, 2 * QB], fp32, name="zrec", tag="zrec")
                nc.scalar.activation(warm[0:1, 0:16], ones_f32[0:1, 0:16], Exp)

                # ---- in-projection ----
                def _inproj_qk(m):
                    for nb in range(nqb):
                        ps = ps_sp.tile([128, 2, QB], fp32, name="sp", tag="sp")[
                            :, 0, :
                        ]
                        for dc in range(8):
                            nc.tensor.matmul(
                                ps[:],
                                wqkT_sb[:, dc, m * 128 : (m + 1) * 128],
                                xT_sb[:, dc, nb * QB : (nb + 1) * QB],
                                start=(dc == 0),
                                stop=(dc == 7),
                            )
                        nc.scalar.activation(
                            qkT_sb[:, m, nb * QB : (nb + 1) * QB],
                            ps[:],
                            Ident,
                            bias=bqk_sb[:, m : m + 1],
                            scale=0.125 if m < 2 else 1.0,
                        )

                def _v_block(lc, vp):
                    # in-proj for v heads [2*vp, 2*vp+1] at key chunk lc,
                    # using the shared pp PSUM bank
                    def _go():
                        ps = ps_pp.tile([128, QB], fp32, name="pp", tag="pp")[
                            :, 0:128
                        ]
                        for dc in range(8):
                            nc.tensor.matmul(
                                ps[:],
                                xT_sb[:, dc, lc * 128 : (lc + 1) * 128],
                                wvT_sb[:, dc, vp * 128 : (vp + 1) * 128],
                                start=(dc == 0),
                                stop=(dc == 7),
                            )
                        nc.vector.tensor_add(
                            v_sb[:, lc, 2 * vp : 2 * vp + 2, 0:64],
                            ps.rearrange("p (h x) -> p h x", h=2),
                            bvr_sb.rearrange("p (h x) -> p h x", h=4)[
                                :, 2 * vp : 2 * vp + 2, :
                            ],
                        )

                    return _go

                def _qk_block(m, nb):
                    def _go():
                        _inproj_qk_one(m, nb)

                    return _go

                def _inproj_qk_one(m, nb):
                    ps = ps_pp.tile([128, QB], fp32, name="pp", tag="pp")
                    for dc in range(8):
                        nc.tensor.matmul(
                            ps[:],
                            wqkT_sb[:, dc, m * 128 : (m + 1) * 128],
                            xT_sb[:, dc, nb * QB : (nb + 1) * QB],
                            start=(dc == 0),
                            stop=(dc == 7),
                        )
                    nc.scalar.activation(
                        qkT_sb[:, m, nb * QB : (nb + 1) * QB],
                        ps[:],
                        Ident,
                        bias=bqk_sb[:, m : m + 1],
                        scale=0.125 if m < 2 else 1.0,
                    )

                # k01 and q01 serial (needed by phase (0,0) immediately)
                _inproj_qk(2)
                _inproj_qk(0)

                # ---- attention phase ----
                nkc_last = nkc - 1

                def _phase(qb, hp, inject):
                    ot_a = ps_ot.tile([65, QB], fp32, name="ot_a", tag="ps_ot")
                    ot_b = ps_ot.tile([65, QB], fp32, name="ot_b", tag="ps_ot")

                    def _emit_av(kc, prob):
                        nc.tensor.matmul(
                            ot_a[:, :],
                            v_sb[:, kc, 2 * hp, :],
                            prob[:, 0, :],
                            start=(kc == 0),
                            stop=(kc == nkc_last),
                        )
                        nc.tensor.matmul(
                            ot_b[:, :],
                            v_sb[:, kc, 2 * hp + 1, :],
                            prob[:, 1, :],
                            start=(kc == 0),
                            stop=(kc == nkc_last),
                        )

                    av_q = []
                    for kc in range(nkc):
                        btab = biasp.tile(
                            [128, 2, QB], bf16, name="btab", tag="bias"
                        )
                        nc.sync.dma_start(btab[:], biasT_d[hp, kc, qb])
                        sp = ps_sp.tile([128, 2, QB], fp32, name="sp", tag="sp")
                        nc.tensor.matmul(
                            sp[:, 0, :],
                            qkT_sb[0:64, 2 + hp, kc * 128 : (kc + 1) * 128],
                            qkT_sb[0:64, hp, qb * QB : (qb + 1) * QB],
                            start=True,
                            stop=True,
                        )
                        nc.tensor.matmul(
                            sp[:, 1, :],
                            qkT_sb[64:128, 2 + hp, kc * 128 : (kc + 1) * 128],
                            qkT_sb[64:128, hp, qb * QB : (qb + 1) * QB],
                            start=True,
                            stop=True,
                        )
                        for item in inject[kc]:
                            item()
                        while len(av_q) > 2:
                            _emit_av(*av_q.pop(0))
                        eq = expp.tile([128, 2, QB], bf16, name="eq", tag="eq")
                        nc.scalar.activation(eq[:, :, :], sp[:, :, :], Exp)
                        prob = probsp.tile(
                            [128, 2, QB], bf16, name="prob", tag="probs"
                        )
                        nc.vector.tensor_mul(prob[:], eq[:], btab[:])
                        av_q.append((kc, prob))
                    for item in av_q:
                        _emit_av(*item)

                    # ---- epilogue closures (run inside the NEXT phase) ----
                    qcols = slice(qb * QB, (qb + 1) * QB)
                    zrec = zrecp.tile([65, 2 * QB], fp32, name="zrec", tag="zrec")
                    otn_odd = zrecp.tile([64, QB], bf16, name="otn_odd", tag="otn_odd")
                    zb = ps_pp.tile([128, QB], fp32, name="zb", tag="pp")

                    def _ep_recip():
                        nc.vector.reciprocal(zrec[64:65, 0:QB], ot_a[64:65, :])
                        nc.vector.reciprocal(zrec[64:65, QB : 2 * QB], ot_b[64:65, :])

                    def _ep_zb():
                        nc.tensor.matmul(
                            zb[0:64, :],
                            ones_f32[64:65, :],
                            zrec[64:65, 0:QB],
                            start=True,
                            stop=True,
                            tile_position=(64, 0),
                        )
                        nc.tensor.matmul(
                            zb[64:128, :],
                            ones_f32[64:65, :],
                            zrec[64:65, QB : 2 * QB],
                            start=True,
                            stop=True,
                            tile_position=(64, 64),
                        )

                    zb_sb = zrecp.tile([128, QB], fp32, name="zb_sb", tag="zb_sb")

                    def _ep_otn():
                        nc.vector.tensor_copy(zb_sb[:], zb[:])
                        nc.vector.tensor_mul(
                            otn2_sb[0:64, hp, qcols], ot_a[0:64, :], zb_sb[0:64, :]
                        )
                        nc.vector.tensor_mul(
                            otn_odd[:, :], ot_b[0:64, :], zb_sb[64:128, :]
                        )
                        nc.sync.dma_start(otn2_sb[64:128, hp, qcols], otn_odd[:, :])

                    return [_ep_recip, _ep_zb, _ep_otn]

                def _op_tile(lc, jbx):
                    def _go():
                        pps = ps_pp.tile([128, QB], fp32, name="pps", tag="pp")
                        for p in range(2):
                            nc.tensor.matmul(
                                pps[:],
                                otn2_sb[:, p, lc * 128 : (lc + 1) * 128],
                                wo2_sb[:, p, jbx * QB : (jbx + 1) * QB],
                                start=(p == 0),
                                stop=(p == 1),
                            )
                        osb = outp.tile([128, QB], bf16, name="osb", tag="osb")
                        nc.vector.tensor_copy(osb[:], pps[:])
                        nc.sync.dma_start(
                            out_d[
                                lc * 128 : (lc + 1) * 128,
                                jbx * QB : (jbx + 1) * QB,
                            ],
                            osb[:],
                        )

                    return _go

                def _outproj_items(qb):
                    return [
                        _op_tile(lc, jbx)
                        for lc in range(qb * (QB // 128), (qb + 1) * (QB // 128))
                        for jbx in range(2)
                    ]

                def _slots(items):
                    """Distribute closures over nkc injection slots in order,
                    at most 2 per slot (front-loaded)."""
                    out = [[] for _ in range(nkc)]
                    for i, it in enumerate(items):
                        out[min(i, nkc - 1)].append(it)
                    return out

                # phase (0,0): v01 blocks doubled at slots 0-7, q23/k23
                # in-proj blocks at slots 8-15
                inj = [[] for _ in range(nkc)]
                for lc in range(nlc):
                    inj[lc // 2].append(_v_block(lc, 0))
                for i, (m, nb) in enumerate(
                    [(1, nb) for nb in range(nqb)] + [(3, nb) for nb in range(nqb)]
                ):
                    inj[8 + i].append(_qk_block(m, nb))
                ep = _phase(0, 0, inj)

                # phase (0,1): epilogue(0,0) at 0-2, v23 packed from slot 3
                inj = [[] for _ in range(nkc)]
                for i, item in enumerate(ep):
                    inj[i].append(item)
                # 16 v23 blocks into slots 3..15: first six doubled at 3-5,
                # the rest at slot lc (always before AV(lc) at slot lc+2)
                for i in range(nlc):
                    inj[3 + i // 2 if i < 6 else i].append(_v_block(i, 1))
                ep = _phase(0, 1, inj)

                for qb in range(1, nqb):
                    inj = _slots(ep + _outproj_items(qb - 1))
                    ep = _phase(qb, 0, inj)
                    inj = _slots(ep)
                    ep = _phase(qb, 1, inj)

                # drain: epilogue(3,1) + outproj(3)
                for item in ep + _outproj_items(nqb - 1):
                    item()

            if loop_n <= 1:
                _emit()
            else:
                with tc.For_i(0, loop_n, 1):
                    _emit()

    nc.compile()
    return nc


def _tile_bias(bias4, Lx=L):
    """[4, Lq, Lk] -> tiled bf16 [2, nkc, nqb, 128, 1024]:
    [...,:512] = head 2hp (S^T layout: k on partitions), [...,512:] = head 2hp+1."""
    nkc, nqb = Lx // 128, Lx // QB
    bT = bias4.transpose(0, 2, 1).reshape(4, nkc, 128, nqb, QB)
    out = np.empty((2, nkc, nqb, 128, 2 * QB), dtype=_BF16)
    for hp in range(2):
        out[hp, :, :, :, 0:QB] = bT[2 * hp].transpose(0, 2, 1, 3).astype(_BF16)
        out[hp, :, :, :, QB:] = bT[2 * hp + 1].transpose(0, 2, 1, 3).astype(_BF16)
    return out


def _shard_inputs(x, key_padding_mask, attn_bias, W_in, b_in, W_out, b_out, Lx=L):
    """Host-side layout prep: slice per core, transpose/cast, exp(bias)*mask."""
    in_maps = []
    W_out_T = np.ascontiguousarray(W_out.T)
    for c in range(8):
        b = c // 4
        h0 = (c % 4) * NHC
        rows_q = slice(h0 * d, (h0 + NHC) * d)
        rows_k = slice(D + h0 * d, D + (h0 + NHC) * d)
        rows_v = slice(2 * D + h0 * d, 2 * D + (h0 + NHC) * d)
        wqk = np.concatenate([W_in[rows_q], W_in[rows_k]], axis=0)  # [512, D]
        wqkT = np.ascontiguousarray(wqk.T).astype(_BF16)
        wvT = np.ascontiguousarray(W_in[rows_v].T).astype(_BF16)
        # [128, 2, D]: pair p -> partitions 0:64 head 2p, 64:128 head 2p+1
        wo2 = np.empty((128, 2, D), dtype=_BF16)
        for p in range(2):
            wo2[0:64, p, :] = W_out_T[(h0 + 2 * p) * d : (h0 + 2 * p + 1) * d]
            wo2[64:128, p, :] = W_out_T[(h0 + 2 * p + 1) * d : (h0 + 2 * p + 2) * d]
        bqk_vec = np.concatenate([b_in[rows_q] / 8.0, b_in[rows_k]]).astype(np.float32)
        bqk = np.ascontiguousarray(bqk_vec.reshape(4, 128).T)
        bvr = np.ascontiguousarray(
            np.broadcast_to(b_in[rows_v].astype(np.float32), (128, 256))
        )
        # exp(bias) with the key-padding mask folded in multiplicatively
        eb = np.exp(attn_bias[b, h0 : h0 + NHC].astype(np.float32))
        eb[:, :, np.asarray(key_padding_mask[b]) == 1] = 0.0
        biasT = _tile_bias(eb)
        xT = np.ascontiguousarray(x[b].T).astype(_BF16)
        in_maps.append(
            {
                "xT": xT,
                "wqkT": wqkT,
                "wvT": wvT,
                "wo2": wo2,
                "bqk": bqk,
                "bvr": bvr,
                "biasT": biasT,
            }
        )
    return in_maps


def kernel(x, key_padding_mask, attn_bias, W_in, b_in, W_out, b_out):
    from concourse.bass_utils import run_bass_kernel_spmd

    if "nc" not in _cached:
        _cached["nc"] = _build_nc()
    nc = _cached["nc"]

    in_maps = _shard_inputs(
        np.asarray(x),
        np.asarray(key_padding_mask),
        np.asarray(attn_bias),
        np.asarray(W_in),
        np.asarray(b_in),
        np.asarray(W_out),
        np.asarray(b_out),
    )
    res = run_bass_kernel_spmd(nc, in_maps, core_ids=list(range(8)))
    out = np.empty((B, L, D), dtype=np.float32)
    b_out32 = np.asarray(b_out).astype(np.float32)
    for b in range(B):
        acc = res.results[4 * b]["partial"].astype(np.float32)
        for c in range(4 * b + 1, 4 * b + 4):
            acc = acc + res.results[c]["partial"].astype(np.float32)
        out[b] = acc + b_out32
    return out
